# revision 3
# baseline (speedup 1.0000x reference)
"""GPT-OSS MoE layer (E=32 experts, top-4, H=I=1024, T=1024 tokens) on 8 TRN2
NeuronCores.

Expert-parallel sharding (4 experts/core). The host computes the router
dispatch (token->expert assignment) and performs the all-to-all gather/
scatter as part of sharding; every MLP FLOP (gate/up proj, SwiGLU, down
proj, bias adds, combine-weight scaling) runs on device.

Device layout keeps tokens in the matmul *free* dimension: per expert e the
kernel computes gu.T = W1_e @ X_e.T accumulated over k-tiles, SwiGLU via the
ACT engine (Silu with per-partition bias), then y.T = W2_e @ h.T, adds b2,
multiplies by the per-token combine weight (broadcast across partitions via
a PE outer product), and DMAs y.T out. Weights stream from HBM exactly once
in [128, 512] chunks alternating across HWDGE/SWDGE queues to reach the
~358 GB/s per-core HBM bound (this problem is memory-regime).
"""

import os
import sys
import types

import numpy as np

NUM_EXPERTS = 32
TOP_K = 4
H = 1024
INTER = 1024
N_CORES = 8
EPC = NUM_EXPERTS // N_CORES  # experts per core
P = 128


def _install_ntff_hook():
    """Best-effort: restore the NTFF profile hook missing from this image so
    trace=True (or BASS_TRACE=1) in run_bass_kernel_spmd can measure HW time."""
    try:
        from antenv.axon_hooks import get_axon_ntff_profile_hook  # noqa: F401

        return
    except ImportError:
        pass
    try:
        from trn_agent_boot.trn_boot import _ntff_profile_via_ctypes

        hook = _ntff_profile_via_ctypes("/opt/axon/libaxon_pjrt.so")
        mod = types.ModuleType("antenv.axon_hooks")
        mod.get_axon_ntff_profile_hook = lambda: hook
        mod.set_axon_ntff_profile_hook = lambda h: None
        sys.modules["antenv.axon_hooks"] = mod
    except Exception:
        pass


_install_ntff_hook()

_NC_CACHE = {}
last_exec_time_ns = None


def _build_nc(C):
    """Build + compile the per-core Bass program for token capacity C."""
    import concourse.mybir as mybir
    import concourse.tile as tile
    from concourse import bacc

    dt = mybir.dt.float32
    AF = mybir.ActivationFunctionType

    nc = bacc.Bacc(trn_type="TRN2")
    xg = nc.dram_tensor("xg", [EPC, H, C], dt, kind="ExternalInput")
    w1p = nc.dram_tensor("w1p", [EPC, H, 2 * INTER], dt, kind="ExternalInput")
    w2t = nc.dram_tensor("w2t", [EPC, INTER, H], dt, kind="ExternalInput")
    b1p = nc.dram_tensor("b1p", [EPC, P, 16], dt, kind="ExternalInput")
    b2p = nc.dram_tensor("b2p", [EPC, P, 8], dt, kind="ExternalInput")
    ce = nc.dram_tensor("ce", [EPC, C], dt, kind="ExternalInput")
    yT = nc.dram_tensor("yT", [EPC, H, C], dt, kind="ExternalOutput")

    KT = H // P  # k tiles per contraction (8)

    with tile.TileContext(nc) as tc:
        with (
            tc.tile_pool(name="const", bufs=1) as const_pool,
            tc.tile_pool(name="xp", bufs=2 * KT) as x_pool,
            tc.tile_pool(name="w1", bufs=6) as w1_pool,
            tc.tile_pool(name="w2", bufs=6) as w2_pool,
            tc.tile_pool(name="hp", bufs=2 * KT) as h_pool,
            tc.tile_pool(name="ev", bufs=3) as ev_pool,
            tc.tile_pool(name="sm", bufs=2) as small_pool,
            tc.tile_pool(name="ps", bufs=1, space="PSUM") as psum_pool,
        ):
            ones = const_pool.tile([1, P], dt)
            nc.vector.memset(ones[:], 1.0)

            for e in range(EPC):
                xt = []
                for k in range(KT):
                    t_ = x_pool.tile([P, C], dt, tag="xt")
                    nc.sync.dma_start(t_[:], xg[e, k * P : (k + 1) * P, :])
                    xt.append(t_)
                b1t = small_pool.tile([P, 16], dt, tag="b1t")
                nc.sync.dma_start(b1t[:], b1p[e])
                b2t = small_pool.tile([P, 8], dt, tag="b2t")
                nc.sync.dma_start(b2t[:], b2p[e])
                ce_row = small_pool.tile([1, C], dt, tag="ce_row")
                nc.sync.dma_start(ce_row[:], ce[e : e + 1, :])
                # broadcast ce across partitions: [128, C] = ones.T @ ce_row
                ce_ps = psum_pool.tile([P, C], dt, tag="g0")
                nc.tensor.matmul(ce_ps[:], ones[:], ce_row[:], start=True, stop=True)
                ce_b = small_pool.tile([P, C], dt, tag="ce_b")
                nc.vector.tensor_copy(ce_b[:], ce_ps[:])

                # ---- gate/up projection + SwiGLU (tokens in free dim) ----
                # w1p columns are packed in pair-blocks [g0 u0 g1 u1 ...]
                h = []
                for mg in range(4):
                    gps = [
                        psum_pool.tile([P, C], dt, tag=t, name=t)
                        for t in ("g0", "u0", "g1", "u1")
                    ]
                    for k in range(KT):
                        wchunk = w1_pool.tile([P, 512], dt, tag="w1c")
                        eng = nc.sync if (k % 2 == 0) else nc.gpsimd
                        eng.dma_start(
                            wchunk[:],
                            w1p[e, k * P : (k + 1) * P, mg * 512 : (mg + 1) * 512],
                        )
                        for j in range(4):
                            nc.tensor.matmul(
                                gps[j][:],
                                wchunk[:, j * P : (j + 1) * P],
                                xt[k][:],
                                start=(k == 0),
                                stop=(k == KT - 1),
                            )
                    for pair in range(2):
                        jg = 4 * mg + 2 * pair  # packed block idx of g half
                        sg = ev_pool.tile([P, C], dt, tag="sg")
                        nc.scalar.activation(
                            sg[:],
                            gps[2 * pair][:],
                            AF.Silu,
                            bias=b1t[:, jg : jg + 1],
                        )
                        us = ev_pool.tile([P, C], dt, tag="us")
                        nc.vector.tensor_scalar_add(
                            us[:], gps[2 * pair + 1][:], b1t[:, jg + 1 : jg + 2]
                        )
                        hm = h_pool.tile([P, C], dt, tag="h")
                        nc.vector.tensor_mul(hm[:], sg[:], us[:])
                        h.append(hm)

                # ---- down projection + bias + combine scale ----
                for m2g in range(2):
                    yps = [
                        psum_pool.tile([P, C], dt, tag=f"y{j}", name=f"y{j}")
                        for j in range(4)
                    ]
                    for k in range(KT):
                        w2chunk = w2_pool.tile([P, 512], dt, tag="w2c")
                        eng = nc.gpsimd if (k % 2 == 0) else nc.sync
                        eng.dma_start(
                            w2chunk[:],
                            w2t[e, k * P : (k + 1) * P, m2g * 512 : (m2g + 1) * 512],
                        )
                        for j in range(4):
                            nc.tensor.matmul(
                                yps[j][:],
                                w2chunk[:, j * P : (j + 1) * P],
                                h[k][:],
                                start=(k == 0),
                                stop=(k == KT - 1),
                            )
                    for j in range(4):
                        m2 = 4 * m2g + j
                        # yo = (y + b2_col) * ce  in one DVE op
                        yo = ev_pool.tile([P, C], dt, tag="yo")
                        nc.vector.scalar_tensor_tensor(
                            yo[:],
                            yps[j][:],
                            b2t[:, m2 : m2 + 1],
                            ce_b[:],
                            mybir.AluOpType.add,
                            mybir.AluOpType.mult,
                        )
                        nc.sync.dma_start(yT[e, m2 * P : (m2 + 1) * P, :], yo[:])

    nc.compile()
    return nc


def _get_nc(C):
    if C not in _NC_CACHE:
        _NC_CACHE[C] = _build_nc(C)
    return _NC_CACHE[C]


def _route(x, wg, bg):
    """Host-side router dispatch: which experts get which tokens, and the
    renormalized combine weights (matches softmax -> top-k -> renorm)."""
    logits = (x.astype(np.float64) @ wg.astype(np.float64).T) + bg.astype(np.float64)
    # top-k by logits == top-k by softmax probs (softmax is monotonic)
    topi = np.argpartition(-logits, TOP_K - 1, axis=1)[:, :TOP_K]  # [T, K]
    topl = np.take_along_axis(logits, topi, axis=1)
    # renormalized combine weight = masked softmax over the top-k logits
    m = topl.max(axis=1, keepdims=True)
    ex = np.exp(topl - m)
    topv = ex / ex.sum(axis=1, keepdims=True)  # [T, K]
    T = x.shape[0]
    combine = np.zeros((T, NUM_EXPERTS), np.float64)
    np.put_along_axis(combine, topi, topv, axis=1)
    idx_per_expert = [np.nonzero(combine[:, e])[0] for e in range(NUM_EXPERTS)]
    return idx_per_expert, combine.astype(np.float32)


def kernel(hidden_states, wg, bg, w1, b1, w2, b2):
    global last_exec_time_ns
    from concourse.bass_utils import run_bass_kernel_spmd

    x = np.ascontiguousarray(hidden_states, np.float32)
    wg = np.asarray(wg, np.float32)
    bg = np.asarray(bg, np.float32)
    w1 = np.asarray(w1, np.float32)
    b1 = np.asarray(b1, np.float32)
    w2 = np.asarray(w2, np.float32)
    b2 = np.asarray(b2, np.float32)
    T = x.shape[0]

    idx_per_expert, combine = _route(x, wg, bg)
    max_n = max(len(ix) for ix in idx_per_expert)
    C = max(16, -(-max_n // 16) * 16)
    assert C <= 512, f"expert capacity {C} exceeds single-matmul free dim"
    nc = _get_nc(C)

    # packed column order for w1.T: pair blocks [g_m | u_m] of 128 channels
    col_order = np.concatenate(
        [
            np.r_[m * P : (m + 1) * P, INTER + m * P : INTER + (m + 1) * P]
            for m in range(INTER // P)
        ]
    )

    in_maps = []
    for c in range(N_CORES):
        xg = np.zeros((EPC, H, C), np.float32)
        ce_arr = np.zeros((EPC, C), np.float32)
        w1p = np.empty((EPC, H, 2 * INTER), np.float32)
        w2t = np.empty((EPC, INTER, H), np.float32)
        b1p = np.empty((EPC, P, 16), np.float32)
        b2p = np.empty((EPC, P, 8), np.float32)
        for je in range(EPC):
            e = EPC * c + je
            ix = idx_per_expert[e]
            n = len(ix)
            if n:
                xg[je, :, :n] = x[ix].T
                ce_arr[je, :n] = combine[ix, e]
            w1p[je] = w1[e].T[:, col_order]
            w2t[je] = w2[e].T
            b1p[je] = b1[e][col_order].reshape(16, P).T
            b2p[je] = b2[e].reshape(8, P).T
        in_maps.append(
            {"xg": xg, "w1p": w1p, "w2t": w2t, "b1p": b1p, "b2p": b2p, "ce": ce_arr}
        )

    trace = bool(int(os.environ.get("KERNEL_TRACE", "0")))
    r = run_bass_kernel_spmd(nc, in_maps, core_ids=list(range(N_CORES)), trace=trace)
    last_exec_time_ns = r.exec_time_ns

    out = np.zeros((T, H), np.float32)
    for c in range(N_CORES):
        yt = r.results[c]["yT"]
        for je in range(EPC):
            e = EPC * c + je
            ix = idx_per_expert[e]
            if len(ix):
                out[ix] += yt[je, :, : len(ix)].T
    return out


# revision 4
# speedup vs baseline: 1.0952x; 1.0952x over previous
"""GPT-OSS MoE layer (E=32 experts, top-4, H=I=1024, T=1024 tokens) on 8 TRN2
NeuronCores.

Expert-parallel sharding (4 experts/core). The host computes the router
dispatch (token->expert assignment) and performs the all-to-all gather/
scatter as part of sharding; every MLP FLOP (gate/up proj, SwiGLU, down
proj, bias adds, combine-weight scaling) runs on device.

Device layout keeps tokens in the matmul *free* dimension: per expert e the
kernel computes gu.T = W1_e @ X_e.T accumulated over k-tiles, SwiGLU via the
ACT engine (Silu with per-partition bias), then y.T = W2_e @ h.T, adds b2,
multiplies by the per-token combine weight (broadcast across partitions via
a PE outer product), and DMAs y.T out. Weights stream from HBM exactly once
in [128, 512] chunks alternating across HWDGE/SWDGE queues to reach the
~358 GB/s per-core HBM bound (this problem is memory-regime).
"""

import os
import sys
import types

import numpy as np

NUM_EXPERTS = 32
TOP_K = 4
H = 1024
INTER = 1024
N_CORES = 8
EPC = NUM_EXPERTS // N_CORES  # experts per core
P = 128


def _install_ntff_hook():
    """Best-effort: restore the NTFF profile hook missing from this image so
    trace=True (or BASS_TRACE=1) in run_bass_kernel_spmd can measure HW time."""
    try:
        from antenv.axon_hooks import get_axon_ntff_profile_hook  # noqa: F401

        return
    except ImportError:
        pass
    try:
        from trn_agent_boot.trn_boot import _ntff_profile_via_ctypes

        hook = _ntff_profile_via_ctypes("/opt/axon/libaxon_pjrt.so")
        mod = types.ModuleType("antenv.axon_hooks")
        mod.get_axon_ntff_profile_hook = lambda: hook
        mod.set_axon_ntff_profile_hook = lambda h: None
        sys.modules["antenv.axon_hooks"] = mod
    except Exception:
        pass


_install_ntff_hook()

_NC_CACHE = {}
last_exec_time_ns = None


def _build_nc(C, TW):
    """Build + compile the per-core Bass program.

    C  = DMA'd token capacity per expert (actual routed max, rounded up)
    TW = matmul moving-dim width (>= 256 so fp32r runs at full rate);
         columns C..TW hold garbage that never reaches the output DMA.
    """
    import concourse.mybir as mybir
    import concourse.tile as tile
    from concourse import bacc

    dt = mybir.dt.float32
    dtr = mybir.dt.float32r
    AF = mybir.ActivationFunctionType

    nc = bacc.Bacc(trn_type="TRN2")
    xg = nc.dram_tensor("xg", [EPC, H, C], dt, kind="ExternalInput")
    w1p = nc.dram_tensor("w1p", [EPC, H, 2 * INTER], dt, kind="ExternalInput")
    w2t = nc.dram_tensor("w2t", [EPC, INTER, H], dt, kind="ExternalInput")
    b1p = nc.dram_tensor("b1p", [EPC, P, 16], dt, kind="ExternalInput")
    b2p = nc.dram_tensor("b2p", [EPC, P, 8], dt, kind="ExternalInput")
    ce = nc.dram_tensor("ce", [EPC, C], dt, kind="ExternalInput")
    yT = nc.dram_tensor("yT", [EPC, H, C], dt, kind="ExternalOutput")

    KT = H // P  # k tiles per contraction (8)

    with tile.TileContext(nc) as tc:
        with (
            tc.tile_pool(name="const", bufs=1) as const_pool,
            tc.tile_pool(name="xp", bufs=2 * KT) as x_pool,
            tc.tile_pool(name="w1", bufs=6) as w1_pool,
            tc.tile_pool(name="w2", bufs=6) as w2_pool,
            tc.tile_pool(name="hp", bufs=2 * KT) as h_pool,
            tc.tile_pool(name="ev", bufs=3) as ev_pool,
            tc.tile_pool(name="sm", bufs=2) as small_pool,
            tc.tile_pool(name="ps", bufs=1, space="PSUM") as psum_pool,
        ):
            ones = const_pool.tile([1, P], dt)
            nc.vector.memset(ones[:], 1.0)

            for e in range(EPC):
                xt = []
                for k in range(KT):
                    t_ = x_pool.tile([P, TW], dtr, tag="xt")
                    nc.sync.dma_start(
                        t_[:, :C], xg[e, k * P : (k + 1) * P, :].bitcast(dtr)
                    )
                    xt.append(t_)
                b1t = small_pool.tile([P, 16], dt, tag="b1t")
                nc.sync.dma_start(b1t[:], b1p[e])
                b2t = small_pool.tile([P, 8], dt, tag="b2t")
                nc.sync.dma_start(b2t[:], b2p[e])
                ce_row = small_pool.tile([1, C], dt, tag="ce_row")
                nc.sync.dma_start(ce_row[:], ce[e : e + 1, :])
                # broadcast ce across partitions: [128, C] = ones.T @ ce_row
                ce_ps = psum_pool.tile([P, TW], dt, tag="g0")
                nc.tensor.matmul(
                    ce_ps[:, :C], ones[:], ce_row[:], start=True, stop=True
                )
                ce_b = small_pool.tile([P, TW], dt, tag="ce_b")
                nc.vector.tensor_copy(ce_b[:, :C], ce_ps[:, :C])

                # ---- gate/up projection + SwiGLU (tokens in free dim) ----
                # w1p columns are packed in pair-blocks [g0 u0 g1 u1 ...]
                h = []
                for mg in range(4):
                    gps = [
                        psum_pool.tile([P, TW], dt, tag=t, name=t)
                        for t in ("g0", "u0", "g1", "u1")
                    ]
                    for k in range(KT):
                        wchunk = w1_pool.tile([P, 512], dtr, tag="w1c")
                        eng = nc.sync if (k % 2 == 0) else nc.gpsimd
                        eng.dma_start(
                            wchunk[:],
                            w1p[
                                e, k * P : (k + 1) * P, mg * 512 : (mg + 1) * 512
                            ].bitcast(dtr),
                        )
                        for j in range(4):
                            nc.tensor.matmul(
                                gps[j][:],
                                wchunk[:, j * P : (j + 1) * P],
                                xt[k][:],
                                start=(k == 0),
                                stop=(k == KT - 1),
                            )
                    for pair in range(2):
                        jg = 4 * mg + 2 * pair  # packed block idx of g half
                        sg = ev_pool.tile([P, TW], dt, tag="sg")
                        nc.scalar.activation(
                            sg[:, :C],
                            gps[2 * pair][:, :C],
                            AF.Silu,
                            bias=b1t[:, jg : jg + 1],
                        )
                        us = ev_pool.tile([P, TW], dt, tag="us")
                        nc.vector.tensor_scalar_add(
                            us[:, :C], gps[2 * pair + 1][:, :C], b1t[:, jg + 1 : jg + 2]
                        )
                        hm = h_pool.tile([P, TW], dtr, tag="h")
                        nc.vector.tensor_mul(hm[:, :C], sg[:, :C], us[:, :C])
                        h.append(hm)

                # ---- down projection + bias + combine scale ----
                for m2g in range(2):
                    yps = [
                        psum_pool.tile([P, TW], dt, tag=f"y{j}", name=f"y{j}")
                        for j in range(4)
                    ]
                    for k in range(KT):
                        w2chunk = w2_pool.tile([P, 512], dtr, tag="w2c")
                        eng = nc.gpsimd if (k % 2 == 0) else nc.sync
                        eng.dma_start(
                            w2chunk[:],
                            w2t[
                                e, k * P : (k + 1) * P, m2g * 512 : (m2g + 1) * 512
                            ].bitcast(dtr),
                        )
                        for j in range(4):
                            nc.tensor.matmul(
                                yps[j][:],
                                w2chunk[:, j * P : (j + 1) * P],
                                h[k][:],
                                start=(k == 0),
                                stop=(k == KT - 1),
                            )
                    for j in range(4):
                        m2 = 4 * m2g + j
                        # yo = (y + b2_col) * ce  in one DVE op
                        yo = ev_pool.tile([P, TW], dt, tag="yo")
                        nc.vector.scalar_tensor_tensor(
                            yo[:, :C],
                            yps[j][:, :C],
                            b2t[:, m2 : m2 + 1],
                            ce_b[:, :C],
                            mybir.AluOpType.add,
                            mybir.AluOpType.mult,
                        )
                        nc.sync.dma_start(yT[e, m2 * P : (m2 + 1) * P, :], yo[:, :C])

    nc.compile()
    return nc


def _get_nc(C, TW):
    if (C, TW) not in _NC_CACHE:
        _NC_CACHE[(C, TW)] = _build_nc(C, TW)
    return _NC_CACHE[(C, TW)]


def _route(x, wg, bg):
    """Host-side router dispatch: which experts get which tokens, and the
    renormalized combine weights (matches softmax -> top-k -> renorm)."""
    logits = (x.astype(np.float64) @ wg.astype(np.float64).T) + bg.astype(np.float64)
    # top-k by logits == top-k by softmax probs (softmax is monotonic)
    topi = np.argpartition(-logits, TOP_K - 1, axis=1)[:, :TOP_K]  # [T, K]
    topl = np.take_along_axis(logits, topi, axis=1)
    # renormalized combine weight = masked softmax over the top-k logits
    m = topl.max(axis=1, keepdims=True)
    ex = np.exp(topl - m)
    topv = ex / ex.sum(axis=1, keepdims=True)  # [T, K]
    T = x.shape[0]
    combine = np.zeros((T, NUM_EXPERTS), np.float64)
    np.put_along_axis(combine, topi, topv, axis=1)
    idx_per_expert = [np.nonzero(combine[:, e])[0] for e in range(NUM_EXPERTS)]
    return idx_per_expert, combine.astype(np.float32)


def kernel(hidden_states, wg, bg, w1, b1, w2, b2):
    global last_exec_time_ns
    from concourse.bass_utils import run_bass_kernel_spmd

    x = np.ascontiguousarray(hidden_states, np.float32)
    wg = np.asarray(wg, np.float32)
    bg = np.asarray(bg, np.float32)
    w1 = np.asarray(w1, np.float32)
    b1 = np.asarray(b1, np.float32)
    w2 = np.asarray(w2, np.float32)
    b2 = np.asarray(b2, np.float32)
    T = x.shape[0]

    idx_per_expert, combine = _route(x, wg, bg)
    max_n = max(len(ix) for ix in idx_per_expert)
    C = max(16, -(-max_n // 16) * 16)
    assert C <= 512, f"expert capacity {C} exceeds single-matmul free dim"
    TW = max(C, 256)  # fp32r matmul runs full-rate only when moving dim >= 256
    nc = _get_nc(C, TW)

    # packed column order for w1.T: pair blocks [g_m | u_m] of 128 channels
    col_order = np.concatenate(
        [
            np.r_[m * P : (m + 1) * P, INTER + m * P : INTER + (m + 1) * P]
            for m in range(INTER // P)
        ]
    )

    in_maps = []
    for c in range(N_CORES):
        xg = np.zeros((EPC, H, C), np.float32)
        ce_arr = np.zeros((EPC, C), np.float32)
        w1p = np.empty((EPC, H, 2 * INTER), np.float32)
        w2t = np.empty((EPC, INTER, H), np.float32)
        b1p = np.empty((EPC, P, 16), np.float32)
        b2p = np.empty((EPC, P, 8), np.float32)
        for je in range(EPC):
            e = EPC * c + je
            ix = idx_per_expert[e]
            n = len(ix)
            if n:
                xg[je, :, :n] = x[ix].T
                ce_arr[je, :n] = combine[ix, e]
            w1p[je] = w1[e].T[:, col_order]
            w2t[je] = w2[e].T
            b1p[je] = b1[e][col_order].reshape(16, P).T
            b2p[je] = b2[e].reshape(8, P).T
        in_maps.append(
            {"xg": xg, "w1p": w1p, "w2t": w2t, "b1p": b1p, "b2p": b2p, "ce": ce_arr}
        )

    trace = bool(int(os.environ.get("KERNEL_TRACE", "0")))
    r = run_bass_kernel_spmd(nc, in_maps, core_ids=list(range(N_CORES)), trace=trace)
    last_exec_time_ns = r.exec_time_ns

    out = np.zeros((T, H), np.float32)
    for c in range(N_CORES):
        yt = r.results[c]["yT"]
        for je in range(EPC):
            e = EPC * c + je
            ix = idx_per_expert[e]
            if len(ix):
                out[ix] += yt[je, :, : len(ix)].T
    return out
